# revision 53
# baseline (speedup 1.0000x reference)
"""Trainium2 Bass kernel for nn_Attention_60567628808865.

Dense transformer attention block (B=4, S=1024, H=4096, NH=32, D=128):
  qkv = x @ W_pack; RoPE(q, k); causal-masked softmax attention; out @ W_o.

Sharding: tensor-parallel over heads across 8 NeuronCores. Each core computes
4 heads end-to-end; the host sums the 8 partial outputs.

The two big projections (QKV, W_o) run as fp8e4 DoubleRow matmuls (0.5
PE-cycles/row while contracting 2x128 K per instruction) with a 3-term hi/lo
decomposition  x*w ~= xh*wh + xh*wl + xl*wh  where hi and lo are e4m3
quantizations at the SAME power-of-2 scale, so all three products accumulate
in one PSUM chain and a single dequant constant folds into the epilogue
(cos/sin tables for q/k, the softmax-reciprocal broadcast for attention, the
PSUM->SBUF copy for the output). Measured end-to-end error of this scheme is
~2e-3, well under the 2e-2 gate; it is 1.33x the f32r matmul rate.

Attention itself stays f32r (contraction is only 128/1024 deep there, so
DoubleRow pays less) with two structural cuts: fully-masked key-blocks above
the causal diagonal are skipped entirely (25% of attention work), and
fully-unmasked interior blocks skip the mask multiply (ACT exp writes the
f32r prob tile directly).

Softmax runs unnormalized (scores are O(1); exp(-1e9)=0), with the
denominator via a ones-vector matmul and applied after PV through a K=1
broadcast matmul whose lhs constant also carries the attention fp8 scale.
RoPE's rotate-half is a partition shift with the sign folded into the
host-built sin table.
"""
import numpy as np

import concourse.bass as bass  # noqa: F401  (AP types come via tile/bacc)
import concourse.tile as tile
from contextlib import ExitStack
from concourse import bacc, mybir
from concourse import bass_utils

F32 = mybir.dt.float32
F32R = mybir.dt.float32r
BF16 = mybir.dt.bfloat16
F8 = mybir.dt.float8e4
AF = mybir.ActivationFunctionType
ALU = mybir.AluOpType
DR = mybir.MatmulPerfMode.DoubleRow

B, S, H, NH = 4, 1024, 4096, 32
D = H // NH          # 128
T = B * S            # 4096 tokens
N_CORES = 8
HC = NH // N_CORES   # 4 heads per core
SCALE = float(1.0 / np.sqrt(D))
ROPE_BASE = 10000.0

TB = 512             # phase-1 token block (512B fp8 DMA chunks: full DMA rate)
NTB = T // TB        # 8
KT = H // 128        # 32 contraction tiles
KH = KT // 2         # 16 per half
NPR = KT // 2        # 16 DoubleRow k-tile pairs
SX, SW = 8.0, 64.0   # fp8 scales: x and W_pack (hi & lo share the scale)
SP = SX * SW         # 512: QKV psum carries 512*value
SA, SWO = 16.0, 64.0  # attention-out and W_o fp8 scales
SO = SA * SWO        # 1024: W_o psum carries 1024*value

_CACHE = {}


def _build_module(phases=("p1", "p2")):
    nc = bacc.Bacc("TRN2", target_bir_lowering=False, debug=False,
                   num_devices=N_CORES)

    xh_d = nc.dram_tensor("xh", [H, T], F8, kind="ExternalInput").ap()
    xl_d = nc.dram_tensor("xl", [H, T], F8, kind="ExternalInput").ap()
    wqkh = nc.dram_tensor("wqkh", [H, 2 * HC * D], F8, kind="ExternalInput").ap()
    wqkl = nc.dram_tensor("wqkl", [H, 2 * HC * D], F8, kind="ExternalInput").ap()
    wvh = nc.dram_tensor("wvh", [H, HC * D], F8, kind="ExternalInput").ap()
    wvl = nc.dram_tensor("wvl", [H, HC * D], F8, kind="ExternalInput").ap()
    woh = nc.dram_tensor("woh", [HC * D, H], F8, kind="ExternalInput").ap()
    wol = nc.dram_tensor("wol", [HC * D, H], F8, kind="ExternalInput").ap()
    cosT = nc.dram_tensor("cosT", [D, T], F32, kind="ExternalInput").ap()
    sinS = nc.dram_tensor("sinS", [D, T], F32, kind="ExternalInput").ap()
    maskT = nc.dram_tensor("maskT", [B, S, S], BF16, kind="ExternalInput").ap()
    out_p = nc.dram_tensor("out_p", [T, H], BF16, kind="ExternalOutput").ap()
    # fp8 constant (value SP/SA) for DoubleRow denominator matmuls: lhsT
    # [128, 2, 128] -> the denominator lands pre-broadcast on 128 partitions
    # and pre-scaled so atf = psav * (1/den) = SA*attn directly
    import ml_dtypes
    ones8 = nc.inline_tensor(
        np.full((128, 2 * 128), SP / SA, ml_dtypes.float8_e4m3), "ones8").ap()
    ones32f = nc.inline_tensor(
        np.full((128, 128), SP / SA, np.float32), "ones32f").ap().bitcast(F32R)
    ebias_t = nc.inline_tensor(
        np.full((128, 1), -5.0 * np.log(2.0), np.float32), "ebias").ap()
    # broadcast constant carries attention dequant 1/SP and fp8 scale SA
    c1 = nc.inline_tensor(np.full((1, 128), SA / SP, np.float32), "c1").ap().bitcast(F32R)
    EXPB = float(-5.0 * np.log(2.0))  # exp bias 2^-5: keeps fp8 prob copies <240

    with tile.TileContext(nc) as tc, \
         nc.allow_low_precision(reason="fp8/tf32 matmuls; verified against reference"):
        with ExitStack() as octx:
            dram = octx.enter_context(tc.tile_pool(name="dram", bufs=1, space="DRAM"))
            cpool = octx.enter_context(tc.tile_pool(name="consts", bufs=1))
            # scratch: qkT rows (pass p, m): [q_2p, k_2p, q_2p+1, k_2p+1]
            qkT_d = dram.tile([2 * HC * D, T], F32R)
            v_d = dram.tile([T, HC * D], F32R)

            prepool = octx.enter_context(tc.tile_pool(name="p2pre", bufs=1))
            pre = {}
            o8 = cpool.tile([128, 2 * 128], F8)
            o1s = cpool.tile([128, 128], F32R)
            eb = cpool.tile([128, 1], F32)
            o1c = cpool.tile([1, 128], F32R)
            consts_emitted = [False]

            def emit_consts():
                # issued after the first critical weight/x tiles so the tiny
                # transfers don't hold HWDGE slots ahead of them
                if consts_emitted[0]:
                    return
                consts_emitted[0] = True
                nc.sync.dma_start(o8[:], ones8[:])
                nc.sync.dma_start(o1s[:], ones32f[:])
                nc.sync.dma_start(eb[:], ebias_t[:])
                nc.sync.dma_start(o1c[:], c1[:])

            # ---------------- Phase 1: QKV projection (fp8 3-term) ----------
            if "p1" in phases:
              with ExitStack() as ctx:
                wpool = ctx.enter_context(tc.tile_pool(name="p1w", bufs=2))
                wvpool = ctx.enter_context(tc.tile_pool(name="p1wv", bufs=1))
                xpool = ctx.enter_context(tc.tile_pool(name="p1x", bufs=2))
                opool = ctx.enter_context(tc.tile_pool(name="p1o", bufs=2))
                cpool1 = ctx.enter_context(tc.tile_pool(name="p1cs", bufs=2))
                rpool1 = ctx.enter_context(tc.tile_pool(name="p1rope", bufs=1))
                pqk = ctx.enter_context(tc.tile_pool(name="p1pqk", bufs=6, space="PSUM"))
                pv = ctx.enter_context(tc.tile_pool(name="p1pv", bufs=2, space="PSUM"))

                def load_x_lv(xs, lv, p, tb):
                    src = xh_d if lv == "h" else xl_d
                    t0 = tb * TB
                    for kh in range(2):
                        xt = xpool.tile([128, KH * TB], F8, tag=f"x{lv}{kh}")
                        nc.sync.dma_start(
                            xt[:].rearrange("p (kk t) -> p kk t", kk=KH),
                            src[kh * 2048:(kh + 1) * 2048, t0:t0 + TB]
                                .rearrange("(kk p) t -> p kk t", p=128))
                        xs[(lv, kh)] = xt

                def load_cos_sin(tb):
                    t0 = tb * TB
                    cos_tb = cpool1.tile([128, TB], F32, tag="cos")
                    nc.sync.dma_start(cos_tb[:], cosT[:, t0:t0 + TB])
                    sin_tb = cpool1.tile([128, TB], F32, tag="sin")
                    nc.sync.dma_start(sin_tb[:], sinS[:, t0:t0 + TB])
                    return cos_tb, sin_tb

                def load_tb_inputs(p, tb):
                    cos_tb, sin_tb = load_cos_sin(tb)
                    xs = {}
                    load_x_lv(xs, "h", p, tb)
                    load_x_lv(xs, "l", p, tb)
                    return cos_tb, sin_tb, xs

                def load_wqk_lv(tiles, lv, p):
                    src = wqkh if lv == "h" else wqkl
                    for kh in range(2):
                        wt = wpool.tile([128, KH * 512], F8, tag=f"wqk{lv}{kh}")
                        nc.sync.dma_start(
                            wt[:].rearrange("p (kk f) -> p kk f", kk=KH),
                            src[kh * KH * 128:(kh + 1) * KH * 128,
                                p * 512:(p + 1) * 512]
                                .rearrange("(kk p) f -> p kk f", p=128))
                        tiles[(lv, kh)] = wt

                def load_wqk(p):
                    tiles = {}
                    load_wqk_lv(tiles, "h", p)
                    load_wqk_lv(tiles, "l", p)
                    return tiles

                def load_wv_lv(tiles, lv):
                    src = wvh if lv == "h" else wvl
                    for kh in range(2):
                        wt = wvpool.tile([128, KH * 512], F8, tag=f"wv{lv}{kh}")
                        nc.sync.dma_start(
                            wt[:].rearrange("p (kk f) -> p kk f", kk=KH),
                            src[kh * KH * 128:(kh + 1) * KH * 128, :]
                                .rearrange("(kk p) f -> p kk f", p=128))
                        tiles[(lv, kh)] = wt

                # startup transfer order tracks first-tb consumption order:
                # chains are term-major (wh*xh over all m, then wl*xh, then
                # wh*xl), so: xh, wqkh, wqkl, cos/sin, xl, wvh, wvl
                xs0 = {}
                wq0 = {}
                wv_t = {}
                for kh in range(2):
                    xt = xpool.tile([128, KH * TB], F8, tag=f"xh{kh}", name="xt0")
                    nc.sync.dma_start(
                        xt[:].rearrange("p (kk t) -> p kk t", kk=KH),
                        xh_d[kh * 2048:(kh + 1) * 2048, 0:TB]
                            .rearrange("(kk p) t -> p kk t", p=128))
                    xs0[("h", kh)] = xt
                    wt = wpool.tile([128, KH * 512], F8, tag=f"wqkh{kh}", name="wt0")
                    nc.sync.dma_start(
                        wt[:].rearrange("p (kk f) -> p kk f", kk=KH),
                        wqkh[kh * KH * 128:(kh + 1) * KH * 128, 0:512]
                            .rearrange("(kk p) f -> p kk f", p=128))
                    wq0[("h", kh)] = wt
                    if kh == 0:
                        emit_consts()
                load_wqk_lv(wq0, "l", 0)
                cs0 = load_cos_sin(0)
                load_x_lv(xs0, "l", 0, 0)
                load_wv_lv(wv_t, "h")   # wv loaded once, full width (both passes)
                tb1_inputs = load_tb_inputs(0, 1)   # ahead of wvl: needed sooner
                load_wv_lv(wv_t, "l")
                inputs_next = (cs0[0], cs0[1], xs0)
                wq_by_pass = {0: wq0}

                for p in range(2):
                    wq_t = wq_by_pass.pop(p)
                    if p == 1:
                        # phase-2 (b0, l0) q/k rows were written in pass 0;
                        # transfer them during pass 1's compute
                        kq0 = prepool.tile([128, 2 * S], F32R, tag="kq0")
                        nc.sync.dma_start(
                            kq0[:].rearrange("p (j t) -> p j t", j=2),
                            qkT_d[0:256, 0:S].rearrange("(j p) t -> p j t", p=128))
                        pre["kq"] = kq0

                    def wqk_pair(lv, pr, m):
                        # [128, 2, 128] stationary AP for k-tile pair pr
                        wt = wq_t[(lv, pr // 8)]
                        prl = pr % 8
                        return wt[:, prl * 1024:(prl + 1) * 1024] \
                            .rearrange("p (sb f) -> p sb f", sb=2) \
                            [:, :, m * 128:(m + 1) * 128]

                    def wv_pair(lv, pr):
                        wt = wv_t[(lv, pr // 8)]
                        prl = pr % 8
                        return wt[:, prl * 1024:(prl + 1) * 1024] \
                            .rearrange("p (sb f) -> p sb f", sb=2) \
                            [:, :, p * 256:(p + 1) * 256]

                    def x_pair(xs, lv, pr, c0=0, c1=TB):
                        xt = xs[(lv, pr // 8)]
                        prl = pr % 8
                        return xt[:, prl * 2 * TB:(prl + 1) * 2 * TB] \
                            .rearrange("p (sb t) -> p sb t", sb=2)[:, :, c0:c1]

                    for tb in range(NTB):
                        t0 = tb * TB
                        cos_tb, sin_tb, xs = inputs_next
                        if p == 0 and tb == 0:
                            inputs_next = tb1_inputs
                        elif tb + 1 < NTB:
                            inputs_next = load_tb_inputs(p, tb + 1)
                        elif p == 0:
                            wq_by_pass[1] = load_wqk(1)
                            inputs_next = load_tb_inputs(1, 0)

                        # 3-term chains, term-major within each chain; for
                        # the very first tile block the chains go term-major
                        # ACROSS all 4 open PSUM chains so the first 64
                        # matmuls need only (wqkh, xh) — minimizes startup
                        # stall while weights/x stream in
                        first_blk = (p == 0 and tb == 0)
                        qs_all = opool.tile([128, 4 * TB], F32R, tag="qs")
                        terms = (("h", "h"), ("l", "h"), ("h", "l"))
                        NPR3 = 4   # q/k xl-term pairs (v chains keep all 16)
                        if first_blk:
                            # kh-major within each term: the first 32 instrs
                            # need only the kh0 x/weight tiles, which land
                            # first in the startup transfer order
                            qps = [pqk.tile([128, TB], F32, tag="qk", name=f"qk{i}")
                                   for i in range(4)]
                            # gather (ti, kh, m, pr) work items in emission
                            # order, then flag the true first/last per chain
                            items = []
                            for ti, (wlv, xlv) in enumerate(terms):
                                for kh in range(2):
                                    if ti < 2:
                                        nhi = 8
                                    else:
                                        nhi = max(0, min(NPR3 - kh * 8, 8))
                                    for m in range(4):
                                        for prl in range(nhi):
                                            items.append((m, wlv, xlv, kh * 8 + prl))
                            counts = [sum(1 for it in items if it[0] == m)
                                      for m in range(4)]
                            seen = [0, 0, 0, 0]
                            for m, wlv, xlv, pr in items:
                                seen[m] += 1
                                nc.tensor.matmul(
                                    qps[m][:], wqk_pair(wlv, pr, m),
                                    x_pair(xs, xlv, pr),
                                    start=(seen[m] == 1),
                                    stop=(seen[m] == counts[m]),
                                    perf_mode=DR)
                        else:
                            qps = []
                            for m in range(4):
                                ps = pqk.tile([128, TB], F32, tag="qk")
                                qps.append(ps)
                                for ti, (wlv, xlv) in enumerate(terms):
                                    npr_t = NPR if ti < 2 else NPR3
                                    for pr in range(npr_t):
                                        nc.tensor.matmul(
                                            ps[:], wqk_pair(wlv, pr, m),
                                            x_pair(xs, xlv, pr),
                                            start=(ti == 0 and pr == 0),
                                            stop=(ti == 2 and pr == npr_t - 1),
                                            perf_mode=DR)
                        for m in range(4):
                            ps = qps[m]
                            # RoPE fused into the epilogue; cos/sin carry 1/SP
                            rot = rpool1.tile([128, TB], F32, tag="rot")
                            nc.vector.tensor_copy(rot[0:64, :], ps[64:128, :])
                            nc.vector.tensor_copy(rot[64:128, :], ps[0:64, :])
                            m1_ = rpool1.tile([128, TB], F32, tag="m1")
                            nc.vector.tensor_tensor(m1_[:], ps[:], cos_tb[:],
                                                    op=ALU.mult)
                            m2_ = rpool1.tile([128, TB], F32, tag="m2")
                            nc.vector.tensor_tensor(m2_[:], rot[:], sin_tb[:],
                                                    op=ALU.mult)
                            nc.vector.tensor_tensor(qs_all[:, m * TB:(m + 1) * TB],
                                                    m1_[:], m2_[:], op=ALU.add)
                        nc.sync.dma_start(
                            qkT_d[p * 512:(p + 1) * 512, t0:t0 + TB]
                                .rearrange("(m pp) t -> pp m t", pp=128),
                            qs_all[:].rearrange("pp (m t) -> pp m t", m=4))

                        vs_all = opool.tile([128, 4 * 256], F32R, tag="vs")
                        for mt in range(4):
                            ps = pv.tile([128, 256], F32, tag="v")
                            for ti, (xlv, wlv) in enumerate(
                                    (("h", "h"), ("h", "l"), ("l", "h"))):
                                for pr in range(NPR):
                                    nc.tensor.matmul(
                                        ps[:],
                                        x_pair(xs, xlv, pr, mt * 128, (mt + 1) * 128),
                                        wv_pair(wlv, pr),
                                        start=(ti == 0 and pr == 0),
                                        stop=(ti == 2 and pr == NPR - 1),
                                        perf_mode=DR)
                            nc.vector.tensor_copy(vs_all[:, mt * 256:(mt + 1) * 256],
                                                  ps[:])
                        nc.sync.dma_start(
                            v_d[t0:t0 + TB, p * 256:(p + 1) * 256]
                                .rearrange("(mt pp) f -> pp mt f", pp=128),
                            vs_all[:].rearrange("pp (mt f) -> pp mt f", mt=4))

            # ---------------- Phase 2+3: attention + W_o ----------------
            if "p2" in phases:
              emit_consts()
              with ExitStack() as ctx:
                wopool = ctx.enter_context(tc.tile_pool(name="p2wo", bufs=1))
                mpool = ctx.enter_context(tc.tile_pool(name="p2m", bufs=2))
                m2pool = ctx.enter_context(tc.tile_pool(name="p2m2", bufs=2))
                tpool = ctx.enter_context(tc.tile_pool(name="p2t", bufs=2))
                epool = ctx.enter_context(tc.tile_pool(name="p2e", bufs=5))
                efpool = ctx.enter_context(tc.tile_pool(name="p2ef", bufs=12))
                apool = ctx.enter_context(tc.tile_pool(name="p2a", bufs=2))
                opool = ctx.enter_context(tc.tile_pool(name="p2o", bufs=3))
                ps_s = ctx.enter_context(tc.tile_pool(name="p2ps", bufs=2, space="PSUM"))
                ps_o = ctx.enter_context(tc.tile_pool(name="p2po", bufs=3, space="PSUM"))
                ps_d = ctx.enter_context(tc.tile_pool(name="p2pd", bufs=1, space="PSUM"))
                ps_av = ctx.enter_context(tc.tile_pool(name="p2pav", bufs=2, space="PSUM"))

                # W_o hi/lo resident; DMAs deferred into b==0's head loop
                woh_a = wopool.tile([128, HC * H], F8, tag="woh")
                wol_a = wopool.tile([128, HC * H], F8, tag="wol")

                def a_pair(a_t, g, m):
                    return a_t[:, (2 * g) * S:(2 * g + 2) * S] \
                        .rearrange("p (sb t) -> p sb t", sb=2) \
                        [:, :, m * 128:(m + 1) * 128]

                def wo_pair(w_t, g, nf):
                    return w_t[:, (2 * g) * H:(2 * g + 2) * H] \
                        .rearrange("p (sb f) -> p sb f", sb=2) \
                        [:, :, nf:nf + 512]

                # deferred denominator + normalize: emitted one attention
                # block later so the fp8 prob copies (DVE/gpsimd) never stall
                # the in-order PE stream
                pending = [None]

                def flush_norm():
                    if pending[0] is None:
                        return
                    (nt, nmt, ef_tiles, ef8_tiles, psav, ath_all, atl_all,
                     asl) = pending[0]
                    pending[0] = None
                    psd = ps_d.tile([128, 512], F32, tag="dbc", name="psd")
                    if nt == 0:
                        for i in range(nmt):
                            nc.tensor.matmul(
                                psd[:], o1s[:], ef_tiles[i][:],
                                start=(i == 0), stop=(i == nmt - 1))
                    else:
                        for i in range(nmt // 2):
                            nc.tensor.matmul(
                                psd[:],
                                o8[:].rearrange("p (sb f) -> p sb f", sb=2),
                                ef8_tiles[i][:].rearrange("p (sb t) -> p sb t", sb=2),
                                start=(i == 0), stop=(i == nmt // 2 - 1),
                                perf_mode=DR)
                    rd = epool.tile([128, 512], F32R, tag="rd", name="rd")
                    nc.vector.reciprocal(rd[:], psd[:])
                    atf = epool.tile([128, 512], F32, tag="atf", name="atf")
                    nc.vector.tensor_tensor(atf[:], psav[:], rd[:], op=ALU.mult)
                    # hi cast on gpsimd, lo residual on DVE (same scale SA)
                    nc.gpsimd.tensor_copy(ath_all[:, asl], atf[:])
                    nc.vector.tensor_tensor(atl_all[:, asl], atf[:],
                                            ath_all[:, asl], op=ALU.subtract)

                def wo_chunk(wb, ath_all, atl_all, m, half):
                    # one (m, half) slice of batch wb's W_o projection
                    # (fp8 3-term DoubleRow, K=512)
                    bs = wb * S
                    os_ = opool.tile([128, 1024], BF16, tag="os", name="os_")
                    for n in range(2):
                        nf = half * 1024 + n * 512
                        if wb == B - 1 and n == 1:
                            pso = ps_av.tile([128, 512], F32, tag="av", name="pso")
                        else:
                            pso = ps_o.tile([128, 512], F32, tag="o", name="pso")
                        idx = 0
                        for g in range(2):
                            for a_t, w_t in ((ath_all, woh_a),
                                             (ath_all, wol_a),
                                             (atl_all, woh_a)):
                                nc.tensor.matmul(
                                    pso[:], a_pair(a_t, g, m),
                                    wo_pair(w_t, g, nf),
                                    start=(idx == 0), stop=(idx == 5),
                                    perf_mode=DR)
                                idx += 1
                        if n == 0:
                            nc.vector.tensor_scalar_mul(
                                os_[:, 0:512], pso[:], 1.0 / SO)
                        else:
                            nc.scalar.mul(os_[:, 512:1024], pso[:], 1.0 / SO)
                    nc.sync.dma_start(
                        out_p[bs + m * 128:bs + (m + 1) * 128,
                              half * 1024:(half + 1) * 1024], os_[:])

                prev_at = None
                wo_queue = []   # deferred (m, half) WO slices of batch b-1,
                                # drained 4 per attention block to fill the
                                # ACT-bound stretches of batch b's attention
                for b in range(B):
                    bs = b * S
                    ath_all = apool.tile([128, HC * S], F8, tag="ath")
                    atl_all = apool.tile([128, HC * S], F8, tag="atl")
                    mq0 = mq1 = None

                    for l in range(HC):
                        rq = (4 * (l // 2) + 2 * (l % 2)) * 128
                        vcol = (l // 2) * 256 + (l % 2) * 128

                        if b == 0 and l == 0:
                            kq = pre["kq"]
                        else:
                            kq = tpool.tile([128, 2 * S], F32R, tag="kqraw")
                            nc.sync.dma_start(
                                kq[:].rearrange("p (j t) -> p j t", j=2),
                                qkT_d[rq:rq + 256, bs:bs + S]
                                    .rearrange("(j p) t -> p j t", p=128))
                        vt_ = tpool.tile([128, 8 * 128], F32R, tag="vt")
                        nc.sync.dma_start(
                            vt_[:].rearrange("p (kt d) -> p kt d", kt=8),
                            v_d[bs:bs + S, vcol:vcol + 128]
                               .rearrange("(kt p) d -> p kt d", p=128))
                        q_rope = kq[:, 0:S]
                        k_rope = kq[:, S:2 * S]
                        if l == 0:
                            # causal mask quadrants (bf16 exp01), after kq/vt
                            # so the first scores are not queued behind them
                            mq0 = m2pool.tile([128, 4 * 512], BF16, tag="mq0")
                            nc.sync.dma_start(
                                mq0[:].rearrange("p (mt t) -> p mt t", mt=4),
                                maskT[b, 0:512, 0:512]
                                    .rearrange("(mt p) t -> p mt t", p=128))
                            mq1 = mpool.tile([128, 4 * 512], BF16, tag="mq1")
                            nc.sync.dma_start(
                                mq1[:].rearrange("p (mt t) -> p mt t", mt=4),
                                maskT[b, 512:1024, 512:1024]
                                    .rearrange("(mt p) t -> p mt t", p=128))
                        if b == 0 and l >= 1:
                            lc = l - 1
                            nc.sync.dma_start(woh_a[:, lc * H:(lc + 1) * H],
                                              woh[lc * 128:(lc + 1) * 128, :])
                            nc.sync.dma_start(wol_a[:, lc * H:(lc + 1) * H],
                                              wol[lc * 128:(lc + 1) * 128, :])
                            if l == 3:
                                nc.sync.dma_start(woh_a[:, 3 * H:4 * H],
                                                  woh[3 * 128:4 * 128, :])
                                nc.sync.dma_start(wol_a[:, 3 * H:4 * H],
                                                  wol[3 * 128:4 * 128, :])

                        for nt in range(2):
                            nmt = 4 * (nt + 1)   # causal: nt=0 needs keys 0:512 only
                            nq = nt * 512
                            psav = ps_av.tile([128, 512], F32, tag="av")
                            ef_tiles = []
                            ef8_tiles = []

                            def pv_step(mt):
                                nc.tensor.matmul(
                                    psav[:], vt_[:, mt * 128:(mt + 1) * 128],
                                    ef_tiles[mt][:],
                                    start=(mt == 0), stop=(mt == nmt - 1))

                            # scores run 2 tiles ahead of PV so exp+mask
                            # latency never stalls the PE; all exps carry a
                            # 2^-5 bias so fp8 prob copies (for the DoubleRow
                            # denominator) stay in range — the scale cancels
                            # between PV numerator and denominator
                            for mt in range(nmt):
                                pss = ps_s.tile([128, 512], F32, tag="s")
                                nc.tensor.matmul(
                                    pss[:], k_rope[:, mt * 128:(mt + 1) * 128],
                                    q_rope[:, nq:nq + 512], start=True, stop=True)
                                interior = (nt == 1 and mt < 4)
                                ef = efpool.tile([128, 512], F32R, tag="ef")
                                if nt == 1:
                                    # fp8 prob copies feed the nt=1 DoubleRow
                                    # denominator (>=512 keys: quantization
                                    # averages out); nt=0 denominators use
                                    # the exact f32r probs instead — early
                                    # tokens attend to few keys and fp8 noise
                                    # would not average there
                                    if mt % 2 == 0:
                                        ef8 = efpool.tile([128, 1024], F8, tag="ef8")
                                        ef8_tiles.append(ef8)
                                    e8sl = ef8_tiles[-1][:, (mt % 2) * 512:(mt % 2 + 1) * 512]
                                if interior:
                                    nc.scalar.activation(ef[:], pss[:], AF.Exp,
                                                         scale=SCALE, bias=eb[:, 0:1])
                                    nc.vector.tensor_copy(e8sl, ef[:])
                                else:
                                    ef0 = epool.tile([128, 512], F32, tag="ef0")
                                    nc.scalar.activation(ef0[:], pss[:], AF.Exp,
                                                         scale=SCALE, bias=eb[:, 0:1])
                                    mq = mq0 if nt == 0 else mq1
                                    msl = mq[:, (mt % 4) * 512:(mt % 4 + 1) * 512]
                                    nc.vector.tensor_tensor(ef[:], ef0[:], msl,
                                                            op=ALU.mult)
                                    if nt == 1:
                                        nc.gpsimd.tensor_tensor(e8sl, ef0[:], msl,
                                                                op=ALU.mult)
                                ef_tiles.append(ef)
                                if mt == 2 and pending[0] is not None:
                                    flush_norm()
                                if mt >= 2:
                                    pv_step(mt - 2)
                            pv_step(nmt - 2)
                            pv_step(nmt - 1)
                            asl = slice(l * S + nq, l * S + nq + 512)
                            pending[0] = (nt, nmt, ef_tiles, ef8_tiles, psav,
                                          ath_all, atl_all, asl)
                            for _ in range(4):
                                if wo_queue:
                                    wo_queue.pop(0)()

                    prev_b, prev_at = b - 1, (ath_all, atl_all)
                    while wo_queue:   # drain any leftover slices of b-1
                        wo_queue.pop(0)()
                    wo_queue = [
                        (lambda m=m, half=half, at=prev_at, wb=b:
                         wo_chunk(wb, at[0], at[1], m, half))
                        for m in range(8) for half in range(4)]
                flush_norm()
                for c in wo_queue:
                    c()
    nc.compile()
    return nc


def _host_prep(hidden_states, W_pack, W_o, attention_mask, position_ids):
    import ml_dtypes
    E4 = ml_dtypes.float8_e4m3
    hidden_states = np.asarray(hidden_states, dtype=np.float32)
    W_pack = np.asarray(W_pack, dtype=np.float32)
    W_o = np.asarray(W_o, dtype=np.float32)
    attention_mask = np.asarray(attention_mask, dtype=np.float32)
    pos = np.asarray(position_ids)

    def q8_hilo(a, scale):
        s = (a * scale).astype(np.float32)
        hi = s.astype(E4)
        lo = (s - hi.astype(np.float32)).astype(E4)
        return np.ascontiguousarray(hi), np.ascontiguousarray(lo)

    xT = np.ascontiguousarray(hidden_states.reshape(T, H).T)
    xh, xl = q8_hilo(xT, SX)
    # exp(mask): softmax mask applied multiplicatively after exp
    maskT = np.ascontiguousarray(
        np.exp(attention_mask[:, 0].transpose(0, 2, 1)).astype(ml_dtypes.bfloat16))

    inv = (1.0 / (ROPE_BASE ** (np.arange(0, D, 2, dtype=np.float64) / D)))
    inv = np.concatenate([inv, inv])                       # [D]
    ang = pos.astype(np.float64).reshape(T)[None, :] * inv[:, None]   # [D, T]
    cosT = (np.cos(ang) / SP).astype(np.float32)
    sinT = (np.sin(ang) / SP).astype(np.float32)
    sinS = sinT.copy()
    sinS[:64] = -sinT[:64]
    cosT = np.ascontiguousarray(cosT)
    sinS = np.ascontiguousarray(sinS)

    # quantize full weight matrices once; per-core maps slice them
    Wp_h, Wp_l = q8_hilo(W_pack, SW)
    Wo_h, Wo_l = q8_hilo(W_o, SWO)

    in_maps = []
    for c in range(N_CORES):
        h0 = c * HC

        def qk_cols(Wq):
            qc = [Wq[:, (h0 + l) * D:(h0 + l + 1) * D] for l in range(HC)]
            kc = [Wq[:, H + (h0 + l) * D:H + (h0 + l + 1) * D] for l in range(HC)]
            return np.ascontiguousarray(np.concatenate(
                [qc[0], kc[0], qc[1], kc[1], qc[2], kc[2], qc[3], kc[3]], axis=1))

        def v_cols(Wq):
            return np.ascontiguousarray(np.concatenate(
                [Wq[:, 2 * H + (h0 + l) * D:2 * H + (h0 + l + 1) * D]
                 for l in range(HC)], axis=1))

        in_maps.append({
            "xh": xh, "xl": xl,
            "wqkh": qk_cols(Wp_h), "wqkl": qk_cols(Wp_l),
            "wvh": v_cols(Wp_h), "wvl": v_cols(Wp_l),
            "woh": np.ascontiguousarray(Wo_h[h0 * D:(h0 + HC) * D, :]),
            "wol": np.ascontiguousarray(Wo_l[h0 * D:(h0 + HC) * D, :]),
            "cosT": cosT, "sinS": sinS, "maskT": maskT,
        })
    return in_maps


def kernel(hidden_states, W_pack, W_o, attention_mask, position_ids):
    if "nc" not in _CACHE:
        _CACHE["nc"] = _build_module()
    nc = _CACHE["nc"]
    in_maps = _host_prep(hidden_states, W_pack, W_o, attention_mask, position_ids)
    res = bass_utils.run_bass_kernel_spmd(nc, in_maps, core_ids=list(range(N_CORES)))
    out = res.results[0]["out_p"].astype(np.float64)
    for c in range(1, N_CORES):
        out += res.results[c]["out_p"]
    return out.reshape(B, S, H).astype(np.float32)
